# revision 1
# baseline (speedup 1.0000x reference)
"""Chamfer-distance (bidirectional 1-NN) Bass kernel for Trainium2.

Problem: B=8 batches of N=M=4096 3-D points. For each batch:
    d[n,m] = ||xyz1[n]-xyz2[m]||^2
    dist1/idx1 = min/argmin over m, dist2/idx2 = min/argmin over n.

Sharding: one batch element per NeuronCore (8 cores), fully independent.

Host-side prep (O(N) marshalling): two [9, 4096] fp32 operand panels
    A = [x1^T (3) ; x1^T**2 (3) ; ones (3)]
    B = [2*x2^T (3) ; -ones (3) ; -(x2^T**2) (3)]
A single PE matmul with lhsT = A-chunk [9,128], rhs = B-tile [9,512] yields
    out[p, j] = 2*sum_k x1k*x2k - sum_k x1k^2 - sum_k x2k^2 = -d[p, j]
directly in PSUM (exact fp32 products, fp32 accumulate).  Swapping operands
(lhsT = B-chunk, rhs = A-tile) yields the transposed matrix, so both
directions reduce along the free axis.

The panels are replicated at partition groups {0, 32, 64, 96} so four row
chunks run CONCURRENTLY in the 128x128 PE array via tile_position row-tiling
(each 32-row strip holds one chunk's K=9 stationary operand; the strips
stream disjoint partition ranges of the shared XBUS).

Reduction: ScalarE casts each 4-bank PSUM group to fp16 in SBUF (its only
job); VectorE then folds each chunk's 4096-wide fp16 row with a 4-level
tensor_tensor max tournament (2x_1p packed mode, 2 elem/lane/cycle) down to
256 cell minima (cell s = positions {s + 256k}), and a tiny max/max_index
exports the top-8 cell ids per row.  The host re-evaluates all 8x16
candidate positions with numpy arithmetic that replicates XLA-CPU's fp32
reference bitwise (fma-chain cross term, verified), so dist and idx match
the jax reference exactly.  The argmin's cell carries the top cell-min up
to fp16 rounding of the final values (~2^-11 relative) plus the PE's fp32
rounding (~2e-6), so the top-8 cell set contains the reference argmin with
overwhelming margin (verified empirically: zero mismatches across seeds).
"""

import numpy as np

import concourse.bass as bass
import concourse.mybir as mybir
from concourse.tile import TileContext

N = 4096  # points per batch in xyz1 / xyz2
P = 128  # partitions
NCHUNKS = N // P  # 32
NQUADS = NCHUNKS // 4  # 8 chunk-quads (4 chunks packed in the PE array)
import os

F32 = mybir.dt.float32
F16 = (
    mybir.dt.bfloat16
    if os.environ.get("CD_KERNEL_RDT", "fp16") == "bf16"
    else mybir.dt.float16
)
DVE_CAST_WINDOWS = tuple(
    int(x) for x in os.environ.get("CD_KERNEL_DVEW", "").split(",") if x != ""
)
RED_MODE = os.environ.get("CD_KERNEL_RED", "tt")  # "tt" tournament | "reduce"
U32 = mybir.dt.uint32
MMW = 512  # fp32 matmul moving-operand max / one PSUM bank
NCELL = 256  # cells per row: cell s = {s + NCELL*k}, k < N//NCELL
CELLK = N // NCELL  # 16 members per cell


def build_nc(reps: int = 1) -> bass.Bass:
    nc = bass.Bass()
    panels_d = nc.dram_tensor("panels", [9, 2 * N], F32, kind="ExternalInput")
    cand1 = nc.dram_tensor("cand1", [N, 8], U32, kind="ExternalOutput")
    cand2 = nc.dram_tensor("cand2", [N, 8], U32, kind="ExternalOutput")

    with TileContext(nc) as tc:
        with (
            tc.tile_pool(name="ext", bufs=1) as ext_pool,
            tc.tile_pool(name="work", bufs=2) as work,
            tc.tile_pool(name="small", bufs=4) as small,
            tc.tile_pool(name="outp", bufs=1) as outp,
            tc.tile_pool(name="psum", bufs=2, space="PSUM") as psum_pool,
        ):
            # Panels replicated at the four 32-partition groups for row-tiling.
            panels = ext_pool.tile([128, 2 * N], F32, tag="panels")
            for g in range(4):
                nc.sync.dma_start(
                    out=panels[32 * g : 32 * g + 9, :], in_=panels_d[:, :]
                )

            for direction in [d for _ in range(reps) for d in (0, 1)]:
                lhs_off = 0 if direction == 0 else N
                rhs_off = N if direction == 0 else 0
                cand_dram = cand1 if direction == 0 else cand2

                idx_sb = outp.tile([P, NCHUNKS * 8], U32, tag=f"idx{direction}")

                for q in range(NQUADS):
                    # dsb4[:, g, :] = fp16(-d) row for chunk 4q+g.
                    dsb4 = work.tile([P, 4, N], F16, tag="dsb4")
                    for w in range(8):  # 512-wide m-windows
                        ps = psum_pool.tile([P, 4 * MMW], F32, tag="ps")
                        for g in range(4):
                            c = 4 * q + g
                            nc.tensor.matmul(
                                ps[:, g * MMW : (g + 1) * MMW],
                                lhsT=panels[
                                    32 * g : 32 * g + 9,
                                    lhs_off + c * P : lhs_off + (c + 1) * P,
                                ],
                                rhs=panels[
                                    32 * g : 32 * g + 9,
                                    rhs_off + w * MMW : rhs_off + (w + 1) * MMW,
                                ],
                                start=True,
                                stop=True,
                                tile_position=(32 * g, 0),
                            )
                        # Cast the whole 4-bank group to 16-bit into the
                        # four chunks' window-w slices (one strided op) on
                        # ScalarE (VectorE absorbs windows listed in
                        # CD_KERNEL_DVEW, default none).
                        if w in DVE_CAST_WINDOWS:
                            nc.vector.tensor_copy(
                                dsb4[:, :, w * MMW : (w + 1) * MMW],
                                ps[:, :].rearrange("p (g j) -> p g j", g=4),
                            )
                        else:
                            nc.scalar.copy(
                                dsb4[:, :, w * MMW : (w + 1) * MMW],
                                ps[:, :].rearrange("p (g j) -> p g j", g=4),
                            )
                    for g in range(4):
                        c = 4 * q + g
                        t4 = small.tile([P, NCELL], F16, tag="t4")
                        if RED_MODE == "reduce":
                            # One segmented reduce: cell s = positions
                            # {s*CELLK + j} (contiguous 16-blocks).
                            nc.vector.tensor_reduce(
                                t4,
                                dsb4[:, g, :].rearrange(
                                    "p (s w) -> p s w", w=CELLK
                                ),
                                axis=mybir.AxisListType.X,
                                op=mybir.AluOpType.max,
                            )
                        else:
                            # 4-level fp16 TT-max tournament: cell s =
                            # positions {s + 256*k}.
                            t1 = small.tile([P, N // 2], F16, tag="t1")
                            nc.vector.tensor_tensor(
                                t1,
                                dsb4[:, g, 0 : N // 2],
                                dsb4[:, g, N // 2 : N],
                                op=mybir.AluOpType.max,
                            )
                            t2 = small.tile([P, N // 4], F16, tag="t2")
                            nc.vector.tensor_tensor(
                                t2,
                                t1[:, 0 : N // 4],
                                t1[:, N // 4 : N // 2],
                                op=mybir.AluOpType.max,
                            )
                            t3 = small.tile([P, N // 8], F16, tag="t3")
                            nc.vector.tensor_tensor(
                                t3,
                                t2[:, 0 : N // 8],
                                t2[:, N // 8 : N // 4],
                                op=mybir.AluOpType.max,
                            )
                            nc.vector.tensor_tensor(
                                t4,
                                t3[:, 0:NCELL],
                                t3[:, NCELL : 2 * NCELL],
                                op=mybir.AluOpType.max,
                            )
                        m8 = small.tile([P, 8], F16, tag="m8")
                        nc.vector.max(out=m8, in_=t4)
                        nc.vector.max_index(
                            idx_sb[:, c * 8 : (c + 1) * 8], m8, t4
                        )

                nc.sync.dma_start(
                    out=cand_dram.rearrange("(c p) j -> p c j", p=P), in_=idx_sb
                )
    _cap_sync_waits(nc)
    return nc


def _cap_sync_waits(nc: bass.Bass, limit: int = 1) -> None:
    """Hardware instruction encodings carry a limited number of sync waits
    (fp32 self-loading Matmult and Activation fail codegen above 1-2).

    Cap every engine instruction at `limit` waits by hoisting the excess onto
    freshly inserted same-engine NoOps directly before it.  Sequencer waits
    are blocking, so an earlier same-engine wait is always sound.
    """
    for f in nc.m.functions:
        for blk in f.blocks:
            insertions = []  # (index, nop)
            for idx, inst in enumerate(blk.instructions):
                si = inst.sync_info
                if si is None:
                    continue
                waits = list(si.on_wait)
                if len(waits) <= limit:
                    continue
                for w in waits[: len(waits) - limit]:
                    nop = mybir.InstNoOp(
                        name=nc.get_next_instruction_name(), ins=[], outs=[]
                    )
                    nop.engine = inst.engine
                    nop.sync_info = mybir.SyncInfo(on_wait=[w], on_update=[])
                    nc.register_instruction(nop)
                    insertions.append((idx, nop))
                si.on_wait = waits[len(waits) - limit :]
                inst.sync_info = si
            for idx, nop in reversed(insertions):
                blk.instructions.insert(idx, nop)


_CACHE: dict = {}


def _get_nc(reps: int = 1) -> bass.Bass:
    if reps not in _CACHE:
        _CACHE[reps] = build_nc(reps)
    return _CACHE[reps]


def make_panels(x1: np.ndarray, x2: np.ndarray):
    """Host-side O(N) marshalling: build the [9, 2N] matmul operand panel."""
    p = np.empty((9, 2 * N), dtype=np.float32)
    x1t = x1.T.astype(np.float32)
    x2t = x2.T.astype(np.float32)
    p[0:3, :N] = x1t
    p[3:6, :N] = x1t * x1t
    p[6:9, :N] = 1.0
    p[0:3, N:] = 2.0 * x2t
    p[3:6, N:] = -1.0
    p[6:9, N:] = -(x2t * x2t)
    return p


def run(xyz1: np.ndarray, xyz2: np.ndarray, reps: int = 1, **spmd_kwargs):
    """Run the SPMD kernel on all batch elements; returns BassKernelResults."""
    from concourse.bass_utils import run_bass_kernel_spmd

    B = xyz1.shape[0]
    in_maps = []
    for b in range(B):
        in_maps.append({"panels": make_panels(xyz1[b], xyz2[b])})
    return run_bass_kernel_spmd(
        _get_nc(reps), in_maps, core_ids=list(range(B)), **spmd_kwargs
    )


def _sq_rows(x: np.ndarray) -> np.ndarray:
    """Replicates jnp.sum(x*x, axis=-1) on XLA-CPU bitwise (fp32)."""
    xx = x * x
    return (xx[:, 0] + xx[:, 1]) + xx[:, 2]


def _refine(xq, xd, sq_q, sq_d, seg):
    """Evaluate reference-bitwise d over candidate segments; min/argmin.

    seg: [N, 8] top cell ids; candidates are the 8*CELLK positions they
    cover (cell s holds positions {s + 256*k}).  Replicates XLA-CPU fp32: cross via an fma chain over the 3
    coords (verified bitwise against the jax reference), then
    d = max((sq_q + sq_d) - 2*cross, 0).  Returns (dist, idx) with
    first-occurrence (smallest index) tie-breaking like jnp.argmin.
    """
    f32, f64 = np.float32, np.float64
    if RED_MODE == "reduce":
        cand = (
            seg[:, :, None] * CELLK + np.arange(CELLK)[None, None, :]
        ).reshape(seg.shape[0], -1)  # contiguous 16-blocks
    else:
        cand = (
            seg[:, :, None] + NCELL * np.arange(CELLK)[None, None, :]
        ).reshape(seg.shape[0], -1)  # mod-256 residue cells
    c = xd[cand]  # [N, 128, 3]
    acc = f32(f64(xq[:, None, 0]) * f64(c[..., 0]))
    acc = f32(f64(xq[:, None, 1]) * f64(c[..., 1]) + f64(acc))
    acc = f32(f64(xq[:, None, 2]) * f64(c[..., 2]) + f64(acc))
    d = (sq_q[:, None] + sq_d[cand]) - f32(2.0) * acc
    d = np.maximum(d, f32(0.0))
    dmin = d.min(axis=1)
    masked = np.where(d == dmin[:, None], cand, np.int64(1) << 40)
    idx = masked.min(axis=1).astype(np.int32)
    return dmin, idx


def postprocess(res, xyz1, xyz2):
    r = res.results
    B = xyz1.shape[0]
    dist1 = np.empty((B, N), np.float32)
    idx1 = np.empty((B, N), np.int32)
    dist2 = np.empty((B, N), np.float32)
    idx2 = np.empty((B, N), np.int32)
    for b in range(B):
        x1, x2 = xyz1[b], xyz2[b]
        sq1, sq2 = _sq_rows(x1), _sq_rows(x2)
        c1 = r[b]["cand1"].astype(np.int64)
        c2 = r[b]["cand2"].astype(np.int64)
        dist1[b], idx1[b] = _refine(x1, x2, sq1, sq2, c1)
        dist2[b], idx2[b] = _refine(x2, x1, sq2, sq1, c2)
    return dist1, idx1, dist2, idx2


def kernel(xyz1, xyz2):
    xyz1 = np.asarray(xyz1, dtype=np.float32)
    xyz2 = np.asarray(xyz2, dtype=np.float32)
    res = run(xyz1, xyz2)
    return postprocess(res, xyz1, xyz2)

